# revision 35
# baseline (speedup 1.0000x reference)
"""Trainium2 Bass kernel for nn_NetworkBasic (2-layer SLAYER SNN), v2.

Batch sharded 2/core across 8 cores. Per layer, all matmul work is f16
with exact-precision hi/lo splits (validated vs reference in numpy):

  conv stage (PE): 3x3 spatial conv fused with the (b,w,t)->(t,h) transpose.
      Binary data chunks [128h, 128(2w x 64t)] are the STATIONARY operand
      (3 w-shifted views of a zero-guarded tile); the moving operand is
      [H_dw_hi | H_dw_lo] banded h-conv matrices. Out: y1T[(w2,t), h'] in
      PSUM = exact conv of binary spikes (H split hi/lo f16).
  evac: y1 = hi_block + lo_block, split to y1hi/y1lo f16 (ACT/Pool/DVE).
  temporal stage (PE): y1T chunks stationary (hi then lo), moving is the
      block-diag temporal matrix T = c*P(srm-psp)@D(2nd-diff) in f16 hi/lo
      (3 matmuls/chunk: hi@Thi, hi@Tlo, lo@Thi). Out: what[h', (w2,t')],
      evac'd (ACT) into the scan buffer = "what pre-added".
  scan (DVE custom op + STT): in-place 2nd-order membrane recurrence,
      y_t = (m<=th) + 2d*m + what  /  m[t+1] += -d^2 * m[t-1].
  spikes: s = (m <= th) f16 -> layer-1 guarded input / output DMA.
"""

import os
import numpy as np

import concourse.bass as bass
import concourse.mybir as mybir
from concourse import bacc, bass_utils
from concourse.tile import TileContext

F32 = mybir.dt.float32
F16 = mybir.dt.float16
AO = mybir.AluOpType

# ---------------- problem constants (hardcoded) ----------------
B_FULL, H, W, T = 16, 128, 64, 64
N_CORES = 8
B_LOC = B_FULL // N_CORES          # 2
BW = B_LOC * W                     # 128 (b,w) columns per time step
SP_FREE = BW * T                   # 8192
WG = W + 2                         # guarded w (zero col at each side)
GFREE = B_LOC * WG * T             # guarded free size 8448
NCHUNK = BW // 2                   # 64 chunks of (2w x 64t)
NWAVE = NCHUNK // 4                # 16 waves of 4 chunks

THETA = (30.0, 50.0)
TAU_SR = (1.0, 2.0)
TAU_REF = (1.0, 2.0)

SCAN_POOL = os.environ.get("KERNEL_SCAN_POOL", "0") == "1"
THRESH_SPLIT = os.environ.get("KERNEL_THRESH_SPLIT", "0") == "1"


def _alpha_kernel(tau, mult, eps):
    vals = []
    for t in np.arange(0.0, float(T), 1.0):
        v = mult * t / tau * np.exp(1.0 - t / tau)
        if abs(v) < eps and t > tau:
            break
        vals.append(v)
    if len(vals) < 2:
        vals.append(0.0)
    return np.asarray(vals, np.float32)


SRM_K = [_alpha_kernel(TAU_SR[i], 1.0, 0.01) for i in range(2)]


def _layer_consts(layer):
    d = float(np.exp(-1.0 / TAU_REF[layer]))
    A = -2.0 * THETA[layer] * np.e / TAU_REF[layer]   # ref[k] = A*k*d^k
    c = 1.0 / (A * d)
    theta_hat = float(np.float32(c * THETA[layer]))
    return d, theta_hat


def _temporal_mat(layer):
    """[64,64] fp64 matrix:  what[t'] = sum_t data[t] * M[t, t']."""
    d, _ = _layer_consts(layer)
    A = -2.0 * THETA[layer] * np.e / TAU_REF[layer]
    c = 1.0 / (A * d)
    kern = SRM_K[layer].astype(np.float64)
    P = np.zeros((T, T))
    for t in range(T):
        for k in range(len(kern)):
            if t + k < T:
                P[t, t + k] = kern[k]
    D = np.zeros((T, T))
    for t in range(T):
        D[t, t] = 1.0
        if t + 1 < T:
            D[t, t + 1] = -2.0 * d
        if t + 2 < T:
            D[t, t + 2] = d * d
    return c * (P @ D)


def _hilo_f16(M):
    hi = M.astype(np.float16)
    lo = (M.astype(np.float32) - hi.astype(np.float32)).astype(np.float16)
    return hi, lo


def _t_blockdiag(layer):
    """w2-major block-diag [2T,2T] f16 hi/lo of the temporal matrix."""
    hi, lo = _hilo_f16(_temporal_mat(layer))
    bhi = np.zeros((2 * T, 2 * T), np.float16)
    blo = np.zeros((2 * T, 2 * T), np.float16)
    for i in (0, 1):
        bhi[i * T:(i + 1) * T, i * T:(i + 1) * T] = hi
        blo[i * T:(i + 1) * T, i * T:(i + 1) * T] = lo
    return bhi, blo


def _h_rhs(w):
    """w: [1,1,3,3] fp32 -> [128, 6*128] f16 moving operand:
    cols (dw, ver[hi,lo], h');  Hm_dw[h, h'] = w[h-h'+1, dw]."""
    out = np.zeros((H, 6 * H), np.float16)
    w = np.asarray(w, np.float32)
    for dwi in range(3):
        m = np.zeros((H, H), np.float32)
        for dh in (-1, 0, 1):
            v = np.float32(w[0, 0, dh + 1, dwi])
            for hp in range(H):
                h = hp + dh
                if 0 <= h < H:
                    m[h, hp] = v
        hi, lo = _hilo_f16(m)
        out[:, (2 * dwi) * H:(2 * dwi + 1) * H] = hi
        out[:, (2 * dwi + 1) * H:(2 * dwi + 2) * H] = lo
    return out


# ---------------- custom DVE op registration ----------------
_SNN_OP = None


def _register_snn_op():
    global _SNN_OP
    if _SNN_OP is not None:
        return _SNN_OP
    import concourse.dve_ops as dve_ops
    from concourse.dve_spec import Spec, Src0, Src1, C0, C1, C2, lower
    from concourse.dve_uop import DveOpSpec

    name = "SNN_STEP_ANT"
    if name in dve_ops._SUB_OPCODE_FOR_NAME:
        _SNN_OP = next(op for op in dve_ops.OPS if op.name == name)
        return _SNN_OP

    # out = (s0 >= in0) + in0*s1 + in1
    body = (C0 >= Src0) + Src0 * C1 + Src1
    spec = Spec(
        body=body,
        reference=lambda in0, in1, s0, s1, imm2: (
            (np.float32(s0) >= in0).astype(np.float32)
            + in0 * np.float32(s1)
            + in1
        ).astype(np.float32),
    )
    row = 1 + len(dve_ops.OPS)
    shas = {}
    for ver in ("v3", "v4"):
        try:
            tmp = DveOpSpec(name=name, opcode=row, uops=lower(spec, ver=ver), rd1_en=True)
            shas[ver] = tmp.sha(ver)
        except Exception:
            pass
    op = dve_ops.DveOp(name, spec, subdim=False, uops_sha=shas)
    dve_ops.OPS.append(op)
    dve_ops._SUB_OPCODE_FOR_NAME[name] = row
    dve_ops.CUSTOM_DVE_SPECS[name] = spec
    _SNN_OP = op
    return op


# ---------------- bass kernel trace ----------------
def trace_kernel(nc, x_d, t_d, h_d, e_d, out_d):
    """x_d: [2,128,64,64] f16 dram; t_d: layer->(thi,tlo) [128,128] f16;
    h_d: layer->[128, 768] f16; out_d: [128, 8192] f16 ((h,(b,w,t))."""
    snn_op = _register_snn_op()

    with TileContext(nc) as tc:
        with (
            tc.tile_pool(name="const", bufs=1) as cpool,
            tc.tile_pool(name="big", bufs=1) as bpool,
            tc.tile_pool(name="ev", bufs=4) as evpool,
            tc.tile_pool(name="pc", bufs=3, space="PSUM") as pc_pool,
            tc.tile_pool(name="pt", bufs=3, space="PSUM") as pt_pool,
        ):
            # constants (emitted after the x DMA below)
            tmats, hmats = {}, {}
            for layer in (0, 1):
                thi = cpool.tile([2 * T, 2 * T], F16, tag=f"thi{layer}")
                tlo = cpool.tile([2 * T, 2 * T], F16, tag=f"tlo{layer}")
                nc.sync.dma_start(out=thi, in_=t_d[layer][0].ap())
                nc.sync.dma_start(out=tlo, in_=t_d[layer][1].ap())
                tmats[layer] = (thi, tlo)
                hm = cpool.tile([H, 6 * H], F16, tag=f"h{layer}")
                nc.sync.dma_start(out=hm, in_=h_d[layer].ap())
                hmats[layer] = hm

            # per-layer Sign-threshold bias consts [128,1]
            thr_bias = {}
            for layer in (0, 1):
                _, th = _layer_consts(layer)
                bval = -float(np.nextafter(np.float32(th), np.float32(np.inf)))
                bt = cpool.tile([H, 1], F32, tag=f"thrb{layer}")
                nc.gpsimd.memset(bt, bval)
                thr_bias[layer] = bt

            # guarded input tiles (layer0 = x, layer1 = s1), (b, wg, t)
            data = {}
            for layer in (0, 1):
                dt_ = bpool.tile([H, GFREE], F16, tag=f"data{layer}")
                dv = dt_[:, :].rearrange("p (b w t) -> p b w t", b=B_LOC, w=WG)
                for b in range(B_LOC):
                    nc.gpsimd.memset(dv[:, b, 0, :], 0.0)
                    nc.gpsimd.memset(dv[:, b, WG - 1, :], 0.0)
                data[layer] = dt_
            d0v = data[0][:, :].rearrange("p (b w t) -> p b w t", b=B_LOC, w=WG)
            for b in range(B_LOC):
                hw_ = W // 2
                nc.sync.dma_start(out=d0v[:, b, 1:1 + hw_, :],
                                  in_=x_d.ap()[b, :, 0:hw_, :])
                nc.sync.dma_start(out=d0v[:, b, 1 + hw_:1 + W, :],
                                  in_=x_d.ap()[b, :, hw_:W, :])

            what = bpool.tile([H, SP_FREE], F32, tag="what")
            wh3 = what[:, :].rearrange("p (t bw) -> p bw t", t=T)
            mh = bpool.tile([H, SP_FREE], F32, tag="mh")
            mh3 = mh[:, :].rearrange("p (t bw) -> p bw t", t=T)
            s2 = bpool.tile([H, SP_FREE], F16, tag="s2")

            for layer in (0, 1):
                d, theta_hat = _layer_consts(layer)
                two_d = float(np.float32(2.0 * d))
                md2 = float(np.float32(-(d * d)))
                thi, tlo = tmats[layer]
                hm = hmats[layer]
                dtile = data[layer]

                # ---- fused conv(+transpose) then temporal, by wave ----
                scopeM = nc.enter_named_scope(f"mm{layer}", False)
                for wv in range(NWAVE):
                    pc = pc_pool.tile([H, 512], F32, tag="pc")
                    for c2 in range(4):
                        ch = wv * 4 + c2
                        b, wp = ch // (W // 2), ch % (W // 2)
                        base = b * (WG * T) + (1 + 2 * wp) * T
                        blk = pc[:, c2 * H:(c2 + 1) * H]
                        for dwi in range(3):
                            lhsT = dtile[:, base + (dwi - 1) * T:
                                         base + (dwi - 1) * T + 2 * T]
                            for v in range(2):
                                nc.tensor.matmul(
                                    blk, lhsT,
                                    hm[:, (2 * dwi + v) * H:
                                       (2 * dwi + v + 1) * H],
                                    start=(dwi == 0 and v == 0),
                                    stop=(dwi == 2 and v == 1),
                                    skip_group_check=True,
                                )
                    # evac: split psum into y1hi/y1lo f16
                    y1hi = evpool.tile([H, 512], F16, tag="y1hi")
                    nc.scalar.copy(y1hi, pc)
                    y1lo = evpool.tile([H, 512], F16, tag="y1lo")
                    nc.vector.scalar_tensor_tensor(
                        y1lo, pc, 0.0, y1hi, AO.bypass, AO.subtract)

                    # temporal: 3 matmuls per chunk into [128,128] block
                    pt = pt_pool.tile([H, 512], F32, tag="pt")
                    for c2 in range(4):
                        lhi = y1hi[:, c2 * H:(c2 + 1) * H]
                        llo = y1lo[:, c2 * H:(c2 + 1) * H]
                        blk = pt[:, c2 * H:(c2 + 1) * H]
                        nc.tensor.matmul(blk, lhi, thi, start=True,
                                         stop=False, skip_group_check=True)
                        nc.tensor.matmul(blk, lhi, tlo, start=False,
                                         stop=False, skip_group_check=True)
                        nc.tensor.matmul(blk, llo, thi, start=False,
                                         stop=True, skip_group_check=True)
                    wslab = what[:, :].rearrange(
                        "p (t c x) -> p c x t", t=T, c=NCHUNK
                    )[:, wv * 4:(wv + 1) * 4, :, :]
                    nc.scalar.copy(
                        wslab,
                        pt[:, :].rearrange("p (c w2 t) -> p c w2 t", c=4, w2=2),
                    )
                nc.leave_named_scope(f"mm{layer}", scopeM[0], False)

                # ---- scan: 1 chained DVE op/step ----
                # off-chain stt (2-step-old input): p[t+1] = md2*m[t-1] + what[t+1]
                # chain:  m[t+1] = (th>=m[t]) + 2d*m[t] + p[t+1]
                scopeS = nc.enter_named_scope(f"scan{layer}", False)
                nc.scalar.copy(mh3[:, :, 0], wh3[:, :, 0])
                ptiles = {}
                for t in range(T - 1):
                    # prep p for step t+2 (reads m[t], which this iter's
                    # custom does not touch -> 2-slot FIFO stagger)
                    if t + 2 <= T - 1:
                        p2 = evpool.tile([H, BW], F32, tag="p")
                        nc.vector.scalar_tensor_tensor(
                            p2, mh3[:, :, t], md2, wh3[:, :, t + 2],
                            AO.mult, AO.add,
                        )
                        ptiles[t + 2] = p2
                    src1 = wh3[:, :, 1] if t == 0 else ptiles.pop(t + 1)
                    nc.vector._custom_dve(
                        snn_op, out=mh3[:, :, t + 1], in0=mh3[:, :, t],
                        in1=src1, s0=theta_hat, s1=two_d,
                    )
                nc.leave_named_scope(f"scan{layer}", scopeS[0], False)

                # ---- spikes on ACT (idle during scan): per 8-t slab,
                #  t1 = Sign(m - nextafter(th)); s = -0.5*t1 + 0.5 ----
                scopeT = nc.enter_named_scope(f"thr{layer}", False)
                d1r = data[1][:, :].rearrange(
                    "p (b w t) -> p t b w", b=B_LOC, w=WG)
                s2r = s2[:, :].rearrange("p (t bw) -> p t bw", t=T)
                for sl in range(T // 8):
                    ts = slice(sl * 8, sl * 8 + 8)
                    mh_sl = mh[:, sl * 1024:(sl + 1) * 1024]
                    if layer == 0:
                        out_ap = d1r[:, ts, :, 1:1 + W]
                    else:
                        out_ap = s2r[:, ts, :]
                    if layer == 0 and sl % 3 == 0:
                        mh_sl3 = mh_sl[:, :].rearrange(
                            "p (t b w) -> p t b w", t=8, b=B_LOC)
                        nc.vector.tensor_scalar(
                            out_ap, mh_sl3, theta_hat, None, AO.is_le)
                        continue
                    t1 = evpool.tile([H, 1024], F32, tag="t1")
                    nc.scalar.activation(
                        t1, mh_sl,
                        mybir.ActivationFunctionType.Sign,
                        bias=thr_bias[layer][:, :])
                    if layer == 0:
                        t13 = t1[:, :].rearrange(
                            "p (t b w) -> p t b w", t=8, b=B_LOC)
                    else:
                        t13 = t1[:, :].rearrange("p (t bw) -> p t bw", t=8)
                    nc.scalar.activation(
                        out_ap, t13,
                        mybir.ActivationFunctionType.Copy,
                        bias=0.5, scale=-0.5)
                    pw = pc_pool.tile([H, 512], F32, tag="pc")
                    if layer == 0:
                        warm_rhs = data[1][:, sl * 16:sl * 16 + 1]
                    else:
                        warm_rhs = s2[:, sl * 16:sl * 16 + 1]
                    nc.tensor.matmul(
                        pw[:, 0:1], tmats[layer][0], warm_rhs,
                        start=True, stop=True, skip_group_check=True)
                if layer == 1:
                    for q in range(4):
                        qs = slice(q * 2048, (q + 1) * 2048)
                        nc.sync.dma_start(out=out_d.ap()[:, qs],
                                          in_=s2[:, qs])
                nc.leave_named_scope(f"thr{layer}", scopeT[0], False)
    return nc


_BUILT = {}


def _build():
    global _BUILT
    key = (SCAN_POOL, THRESH_SPLIT)
    if key in _BUILT:
        return _BUILT[key]
    nc = bacc.Bacc("TRN2", debug=False)
    x_d = nc.dram_tensor("x", [B_LOC, H, W, T], F16, kind="ExternalInput")
    t_d, h_d = {}, {}
    for layer in (0, 1):
        t_d[layer] = (
            nc.dram_tensor(f"t{layer}hi", [2 * T, 2 * T], F16, kind="ExternalInput"),
            nc.dram_tensor(f"t{layer}lo", [2 * T, 2 * T], F16, kind="ExternalInput"),
        )
        h_d[layer] = nc.dram_tensor(f"h{layer}", [H, 6 * H], F16, kind="ExternalInput")
    e_d = {}
    for layer in (0, 1):
        e_d[layer] = nc.dram_tensor(f"e{layer}", [H, H], F32, kind="ExternalInput")
    out_d = nc.dram_tensor("out", [H, SP_FREE], F16, kind="ExternalOutput")
    trace_kernel(nc, x_d, t_d, h_d, e_d, out_d)
    nc.compile()
    _BUILT[key] = nc
    return nc


def _host_inputs(conv1_w, conv2_w):
    ins = {}
    for layer, w in ((0, conv1_w), (1, conv2_w)):
        bhi, blo = _t_blockdiag(layer)
        ins[f"t{layer}hi"] = bhi
        ins[f"t{layer}lo"] = blo
        ins[f"h{layer}"] = _h_rhs(w)
        d, _ = _layer_consts(layer)
        md2 = np.float32(-(d * d))
        ins[f"e{layer}"] = (np.eye(H, dtype=np.float32) * md2)
    return ins


def kernel(spikeInput, conv1_w, conv2_w):
    x = np.asarray(spikeInput, np.float32).reshape(B_FULL, H, W, T)
    x16 = np.ascontiguousarray(x.astype(np.float16))
    common = _host_inputs(conv1_w, conv2_w)
    nc = _build()
    in_maps = []
    for c in range(N_CORES):
        m = dict(common)
        m["x"] = np.ascontiguousarray(x16[c * B_LOC:(c + 1) * B_LOC])
        in_maps.append(m)
    res = bass_utils.run_bass_kernel_spmd(nc, in_maps, core_ids=list(range(N_CORES)))
    # out per core: [h, (b, w, t)] f16 -> [b, h, w, t] f32
    outs = []
    for r in res.results:
        o = r["out"].reshape(H, T, B_LOC, W)
        outs.append(o.transpose(2, 0, 3, 1))
    out = np.concatenate(outs, axis=0).astype(np.float32)
    return np.ascontiguousarray(out)


# revision 36
# speedup vs baseline: 1.0463x; 1.0463x over previous
"""Trainium2 Bass kernel for nn_NetworkBasic (2-layer SLAYER SNN), v2.

Batch sharded 2/core across 8 cores. Per layer, all matmul work is f16
with exact-precision hi/lo splits (validated vs reference in numpy):

  conv stage (PE): 3x3 spatial conv fused with the (b,w,t)->(t,h) transpose.
      Binary data chunks [128h, 128(2w x 64t)] are the STATIONARY operand
      (3 w-shifted views of a zero-guarded tile); the moving operand is
      [H_dw_hi | H_dw_lo] banded h-conv matrices. Out: y1T[(w2,t), h'] in
      PSUM = exact conv of binary spikes (H split hi/lo f16).
  evac: y1 = hi_block + lo_block, split to y1hi/y1lo f16 (ACT/Pool/DVE).
  temporal stage (PE): y1T chunks stationary (hi then lo), moving is the
      block-diag temporal matrix T = c*P(srm-psp)@D(2nd-diff) in f16 hi/lo
      (3 matmuls/chunk: hi@Thi, hi@Tlo, lo@Thi). Out: what[h', (w2,t')],
      evac'd (ACT) into the scan buffer = "what pre-added".
  scan (DVE custom op + STT): in-place 2nd-order membrane recurrence,
      y_t = (m<=th) + 2d*m + what  /  m[t+1] += -d^2 * m[t-1].
  spikes: s = (m <= th) f16 -> layer-1 guarded input / output DMA.
"""

import os
import numpy as np

import concourse.bass as bass
import concourse.mybir as mybir
from concourse import bacc, bass_utils
from concourse.tile import TileContext

F32 = mybir.dt.float32
F16 = mybir.dt.float16
AO = mybir.AluOpType

# ---------------- problem constants (hardcoded) ----------------
B_FULL, H, W, T = 16, 128, 64, 64
N_CORES = 8
B_LOC = B_FULL // N_CORES          # 2
BW = B_LOC * W                     # 128 (b,w) columns per time step
SP_FREE = BW * T                   # 8192
WG = W + 2                         # guarded w (zero col at each side)
GFREE = B_LOC * WG * T             # guarded free size 8448
NCHUNK = BW // 2                   # 64 chunks of (2w x 64t)
NWAVE = NCHUNK // 4                # 16 waves of 4 chunks

THETA = (30.0, 50.0)
TAU_SR = (1.0, 2.0)
TAU_REF = (1.0, 2.0)

SCAN_POOL = os.environ.get("KERNEL_SCAN_POOL", "0") == "1"
THRESH_SPLIT = os.environ.get("KERNEL_THRESH_SPLIT", "0") == "1"


def _alpha_kernel(tau, mult, eps):
    vals = []
    for t in np.arange(0.0, float(T), 1.0):
        v = mult * t / tau * np.exp(1.0 - t / tau)
        if abs(v) < eps and t > tau:
            break
        vals.append(v)
    if len(vals) < 2:
        vals.append(0.0)
    return np.asarray(vals, np.float32)


SRM_K = [_alpha_kernel(TAU_SR[i], 1.0, 0.01) for i in range(2)]


def _layer_consts(layer):
    d = float(np.exp(-1.0 / TAU_REF[layer]))
    A = -2.0 * THETA[layer] * np.e / TAU_REF[layer]   # ref[k] = A*k*d^k
    c = 1.0 / (A * d)
    theta_hat = float(np.float32(c * THETA[layer]))
    return d, theta_hat


def _temporal_mat(layer):
    """[64,64] fp64 matrix:  what[t'] = sum_t data[t] * M[t, t']."""
    d, _ = _layer_consts(layer)
    A = -2.0 * THETA[layer] * np.e / TAU_REF[layer]
    c = 1.0 / (A * d)
    kern = SRM_K[layer].astype(np.float64)
    P = np.zeros((T, T))
    for t in range(T):
        for k in range(len(kern)):
            if t + k < T:
                P[t, t + k] = kern[k]
    D = np.zeros((T, T))
    for t in range(T):
        D[t, t] = 1.0
        if t + 1 < T:
            D[t, t + 1] = -2.0 * d
        if t + 2 < T:
            D[t, t + 2] = d * d
    return c * (P @ D)


def _hilo_f16(M):
    hi = M.astype(np.float16)
    lo = (M.astype(np.float32) - hi.astype(np.float32)).astype(np.float16)
    return hi, lo


def _t_blockdiag(layer):
    """w2-major block-diag [2T,2T] f16 hi/lo of the temporal matrix."""
    hi, lo = _hilo_f16(_temporal_mat(layer))
    bhi = np.zeros((2 * T, 2 * T), np.float16)
    blo = np.zeros((2 * T, 2 * T), np.float16)
    for i in (0, 1):
        bhi[i * T:(i + 1) * T, i * T:(i + 1) * T] = hi
        blo[i * T:(i + 1) * T, i * T:(i + 1) * T] = lo
    return bhi, blo


def _h_rhs(w):
    """w: [1,1,3,3] fp32 -> [128, 6*128] f16 moving operand:
    cols (dw, ver[hi,lo], h');  Hm_dw[h, h'] = w[h-h'+1, dw]."""
    out = np.zeros((H, 6 * H), np.float16)
    w = np.asarray(w, np.float32)
    for dwi in range(3):
        m = np.zeros((H, H), np.float32)
        for dh in (-1, 0, 1):
            v = np.float32(w[0, 0, dh + 1, dwi])
            for hp in range(H):
                h = hp + dh
                if 0 <= h < H:
                    m[h, hp] = v
        hi, lo = _hilo_f16(m)
        out[:, (2 * dwi) * H:(2 * dwi + 1) * H] = hi
        out[:, (2 * dwi + 1) * H:(2 * dwi + 2) * H] = lo
    return out


# ---------------- custom DVE op registration ----------------
_SNN_OP = None


def _register_snn_op():
    global _SNN_OP
    if _SNN_OP is not None:
        return _SNN_OP
    import concourse.dve_ops as dve_ops
    from concourse.dve_spec import Spec, Src0, Src1, C0, C1, C2, lower
    from concourse.dve_uop import DveOpSpec

    name = "SNN_STEP_ANT"
    if name in dve_ops._SUB_OPCODE_FOR_NAME:
        _SNN_OP = next(op for op in dve_ops.OPS if op.name == name)
        return _SNN_OP

    # out = (s0 >= in0) + in0*s1 + in1
    body = (C0 >= Src0) + Src0 * C1 + Src1
    spec = Spec(
        body=body,
        reference=lambda in0, in1, s0, s1, imm2: (
            (np.float32(s0) >= in0).astype(np.float32)
            + in0 * np.float32(s1)
            + in1
        ).astype(np.float32),
    )
    row = 1 + len(dve_ops.OPS)
    shas = {}
    for ver in ("v3", "v4"):
        try:
            tmp = DveOpSpec(name=name, opcode=row, uops=lower(spec, ver=ver), rd1_en=True)
            shas[ver] = tmp.sha(ver)
        except Exception:
            pass
    op = dve_ops.DveOp(name, spec, subdim=False, uops_sha=shas)
    dve_ops.OPS.append(op)
    dve_ops._SUB_OPCODE_FOR_NAME[name] = row
    dve_ops.CUSTOM_DVE_SPECS[name] = spec
    _SNN_OP = op
    return op


# ---------------- bass kernel trace ----------------
def trace_kernel(nc, x_d, t_d, h_d, e_d, out_d):
    """x_d: [2,128,64,64] f16 dram; t_d: layer->(thi,tlo) [128,128] f16;
    h_d: layer->[128, 768] f16; out_d: [128, 8192] f16 ((h,(b,w,t))."""
    snn_op = _register_snn_op()

    with TileContext(nc) as tc:
        with (
            tc.tile_pool(name="const", bufs=1) as cpool,
            tc.tile_pool(name="big", bufs=1) as bpool,
            tc.tile_pool(name="ev", bufs=4) as evpool,
            tc.tile_pool(name="pc", bufs=3, space="PSUM") as pc_pool,
            tc.tile_pool(name="pt", bufs=3, space="PSUM") as pt_pool,
        ):
            # constants (emitted after the x DMA below)
            tmats, hmats = {}, {}
            for layer in (0, 1):
                thi = cpool.tile([2 * T, 2 * T], F16, tag=f"thi{layer}")
                tlo = cpool.tile([2 * T, 2 * T], F16, tag=f"tlo{layer}")
                nc.sync.dma_start(out=thi, in_=t_d[layer][0].ap())
                nc.sync.dma_start(out=tlo, in_=t_d[layer][1].ap())
                tmats[layer] = (thi, tlo)
                hm = cpool.tile([H, 6 * H], F16, tag=f"h{layer}")
                nc.sync.dma_start(out=hm, in_=h_d[layer].ap())
                hmats[layer] = hm

            # per-layer Sign-threshold bias consts [128,1]
            thr_bias = {}
            for layer in (0, 1):
                _, th = _layer_consts(layer)
                bval = -float(np.nextafter(np.float32(th), np.float32(np.inf)))
                bt = cpool.tile([H, 1], F32, tag=f"thrb{layer}")
                nc.gpsimd.memset(bt, bval)
                thr_bias[layer] = bt

            # guarded input tiles (layer0 = x, layer1 = s1), (b, wg, t)
            data = {}
            for layer in (0, 1):
                dt_ = bpool.tile([H, GFREE], F16, tag=f"data{layer}")
                dv = dt_[:, :].rearrange("p (b w t) -> p b w t", b=B_LOC, w=WG)
                for b in range(B_LOC):
                    nc.gpsimd.memset(dv[:, b, 0, :], 0.0)
                    nc.gpsimd.memset(dv[:, b, WG - 1, :], 0.0)
                data[layer] = dt_
            d0v = data[0][:, :].rearrange("p (b w t) -> p b w t", b=B_LOC, w=WG)
            for b in range(B_LOC):
                hw_ = W // 2
                nc.sync.dma_start(out=d0v[:, b, 1:1 + hw_, :],
                                  in_=x_d.ap()[b, :, 0:hw_, :])
                nc.sync.dma_start(out=d0v[:, b, 1 + hw_:1 + W, :],
                                  in_=x_d.ap()[b, :, hw_:W, :])

            what = bpool.tile([H, SP_FREE], F32, tag="what")
            wh3 = what[:, :].rearrange("p (t bw) -> p bw t", t=T)
            mh = bpool.tile([H, SP_FREE], F32, tag="mh")
            mh3 = mh[:, :].rearrange("p (t bw) -> p bw t", t=T)
            s2 = bpool.tile([H, SP_FREE], F16, tag="s2")

            for layer in (0, 1):
                d, theta_hat = _layer_consts(layer)
                two_d = float(np.float32(2.0 * d))
                md2 = float(np.float32(-(d * d)))
                thi, tlo = tmats[layer]
                hm = hmats[layer]
                dtile = data[layer]

                # ---- fused conv(+transpose) then temporal, by wave ----
                scopeM = nc.enter_named_scope(f"mm{layer}", False)
                for wv in range(NWAVE):
                    pc = pc_pool.tile([H, 512], F32, tag="pc")
                    for c2 in range(4):
                        ch = wv * 4 + c2
                        b, wp = ch // (W // 2), ch % (W // 2)
                        base = b * (WG * T) + (1 + 2 * wp) * T
                        blk = pc[:, c2 * H:(c2 + 1) * H]
                        for dwi in range(3):
                            lhsT = dtile[:, base + (dwi - 1) * T:
                                         base + (dwi - 1) * T + 2 * T]
                            for v in range(2):
                                nc.tensor.matmul(
                                    blk, lhsT,
                                    hm[:, (2 * dwi + v) * H:
                                       (2 * dwi + v + 1) * H],
                                    start=(dwi == 0 and v == 0),
                                    stop=(dwi == 2 and v == 1),
                                    skip_group_check=True,
                                )
                    # evac: split psum into y1hi/y1lo f16
                    y1hi = evpool.tile([H, 512], F16, tag="y1hi")
                    nc.scalar.copy(y1hi, pc)
                    y1lo = evpool.tile([H, 512], F16, tag="y1lo")
                    nc.vector.scalar_tensor_tensor(
                        y1lo, pc, 0.0, y1hi, AO.bypass, AO.subtract)

                    # temporal: 3 matmuls per chunk into [128,128] block
                    pt = pt_pool.tile([H, 512], F32, tag="pt")
                    for c2 in range(4):
                        lhi = y1hi[:, c2 * H:(c2 + 1) * H]
                        llo = y1lo[:, c2 * H:(c2 + 1) * H]
                        blk = pt[:, c2 * H:(c2 + 1) * H]
                        nc.tensor.matmul(blk, lhi, thi, start=True,
                                         stop=False, skip_group_check=True)
                        nc.tensor.matmul(blk, lhi, tlo, start=False,
                                         stop=False, skip_group_check=True)
                        nc.tensor.matmul(blk, llo, thi, start=False,
                                         stop=True, skip_group_check=True)
                    wslab = what[:, :].rearrange(
                        "p (t c x) -> p c x t", t=T, c=NCHUNK
                    )[:, wv * 4:(wv + 1) * 4, :, :]
                    nc.scalar.copy(
                        wslab,
                        pt[:, :].rearrange("p (c w2 t) -> p c w2 t", c=4, w2=2),
                    )
                nc.leave_named_scope(f"mm{layer}", scopeM[0], False)

                # ---- scan: 1 chained DVE op/step ----
                # off-chain stt (2-step-old input): p[t+1] = md2*m[t-1] + what[t+1]
                # chain:  m[t+1] = (th>=m[t]) + 2d*m[t] + p[t+1]
                scopeS = nc.enter_named_scope(f"scan{layer}", False)
                nc.scalar.copy(mh3[:, :, 0], wh3[:, :, 0])
                ptiles = {}
                for t in range(T - 1):
                    # prep p for step t+2 (reads m[t], which this iter's
                    # custom does not touch -> 2-slot FIFO stagger)
                    if t + 2 <= T - 1:
                        p2 = evpool.tile([H, BW], F32, tag="p")
                        nc.vector.scalar_tensor_tensor(
                            p2, mh3[:, :, t], md2, wh3[:, :, t + 2],
                            AO.mult, AO.add,
                        )
                        ptiles[t + 2] = p2
                    src1 = wh3[:, :, 1] if t == 0 else ptiles.pop(t + 1)
                    nc.vector._custom_dve(
                        snn_op, out=mh3[:, :, t + 1], in0=mh3[:, :, t],
                        in1=src1, s0=theta_hat, s1=two_d,
                    )
                nc.leave_named_scope(f"scan{layer}", scopeS[0], False)

                # ---- spikes on ACT (idle during scan): per 8-t slab,
                #  t1 = Sign(m - nextafter(th)); s = -0.5*t1 + 0.5 ----
                scopeT = nc.enter_named_scope(f"thr{layer}", False)
                d1r = data[1][:, :].rearrange(
                    "p (b w t) -> p t b w", b=B_LOC, w=WG)
                s2r = s2[:, :].rearrange("p (t bw) -> p t bw", t=T)
                for sl in range(T // 8):
                    ts = slice(sl * 8, sl * 8 + 8)
                    mh_sl = mh[:, sl * 1024:(sl + 1) * 1024]
                    if layer == 0:
                        out_ap = d1r[:, ts, :, 1:1 + W]
                    else:
                        out_ap = s2r[:, ts, :]
                    if layer == 0 and sl in (2, 5, 7):
                        mh_sl3 = mh_sl[:, :].rearrange(
                            "p (t b w) -> p t b w", t=8, b=B_LOC)
                        nc.vector.tensor_scalar(
                            out_ap, mh_sl3, theta_hat, None, AO.is_le)
                        continue
                    if layer == 1 and sl == 7:
                        nc.vector.tensor_scalar(
                            out_ap, mh_sl[:, :].rearrange(
                                "p (t bw) -> p t bw", t=8),
                            theta_hat, None, AO.is_le)
                        continue
                    t1 = evpool.tile([H, 1024], F32, tag="t1")
                    nc.scalar.activation(
                        t1, mh_sl,
                        mybir.ActivationFunctionType.Sign,
                        bias=thr_bias[layer][:, :])
                    if layer == 0:
                        t13 = t1[:, :].rearrange(
                            "p (t b w) -> p t b w", t=8, b=B_LOC)
                    else:
                        t13 = t1[:, :].rearrange("p (t bw) -> p t bw", t=8)
                    nc.scalar.activation(
                        out_ap, t13,
                        mybir.ActivationFunctionType.Copy,
                        bias=0.5, scale=-0.5)
                    pw = pc_pool.tile([H, 512], F32, tag="pc")
                    if layer == 0:
                        warm_rhs = data[1][:, sl * 16:sl * 16 + 1]
                    else:
                        warm_rhs = s2[:, sl * 16:sl * 16 + 1]
                    nc.tensor.matmul(
                        pw[:, 0:1], tmats[layer][0], warm_rhs,
                        start=True, stop=True, skip_group_check=True)
                if layer == 1:
                    for q in range(4):
                        qs = slice(q * 2048, (q + 1) * 2048)
                        nc.sync.dma_start(out=out_d.ap()[:, qs],
                                          in_=s2[:, qs])
                nc.leave_named_scope(f"thr{layer}", scopeT[0], False)
    return nc


_BUILT = {}


def _build():
    global _BUILT
    key = (SCAN_POOL, THRESH_SPLIT)
    if key in _BUILT:
        return _BUILT[key]
    nc = bacc.Bacc("TRN2", debug=False)
    x_d = nc.dram_tensor("x", [B_LOC, H, W, T], F16, kind="ExternalInput")
    t_d, h_d = {}, {}
    for layer in (0, 1):
        t_d[layer] = (
            nc.dram_tensor(f"t{layer}hi", [2 * T, 2 * T], F16, kind="ExternalInput"),
            nc.dram_tensor(f"t{layer}lo", [2 * T, 2 * T], F16, kind="ExternalInput"),
        )
        h_d[layer] = nc.dram_tensor(f"h{layer}", [H, 6 * H], F16, kind="ExternalInput")
    e_d = {}
    for layer in (0, 1):
        e_d[layer] = nc.dram_tensor(f"e{layer}", [H, H], F32, kind="ExternalInput")
    out_d = nc.dram_tensor("out", [H, SP_FREE], F16, kind="ExternalOutput")
    trace_kernel(nc, x_d, t_d, h_d, e_d, out_d)
    nc.compile()
    _BUILT[key] = nc
    return nc


def _host_inputs(conv1_w, conv2_w):
    ins = {}
    for layer, w in ((0, conv1_w), (1, conv2_w)):
        bhi, blo = _t_blockdiag(layer)
        ins[f"t{layer}hi"] = bhi
        ins[f"t{layer}lo"] = blo
        ins[f"h{layer}"] = _h_rhs(w)
        d, _ = _layer_consts(layer)
        md2 = np.float32(-(d * d))
        ins[f"e{layer}"] = (np.eye(H, dtype=np.float32) * md2)
    return ins


def kernel(spikeInput, conv1_w, conv2_w):
    x = np.asarray(spikeInput, np.float32).reshape(B_FULL, H, W, T)
    x16 = np.ascontiguousarray(x.astype(np.float16))
    common = _host_inputs(conv1_w, conv2_w)
    nc = _build()
    in_maps = []
    for c in range(N_CORES):
        m = dict(common)
        m["x"] = np.ascontiguousarray(x16[c * B_LOC:(c + 1) * B_LOC])
        in_maps.append(m)
    res = bass_utils.run_bass_kernel_spmd(nc, in_maps, core_ids=list(range(N_CORES)))
    # out per core: [h, (b, w, t)] f16 -> [b, h, w, t] f32
    outs = []
    for r in res.results:
        o = r["out"].reshape(H, T, B_LOC, W)
        outs.append(o.transpose(2, 0, 3, 1))
    out = np.concatenate(outs, axis=0).astype(np.float32)
    return np.ascontiguousarray(out)
